# revision 6
# baseline (speedup 1.0000x reference)
"""Trainium2 Bass kernel for the CIN block (v2):
out[b,o,k] = sum_{h,m} W[o, h*M+m] * xl[b,h,k] * x0[b,m,k] + bias[o]

Strategy (data-parallel over batch across 8 cores, 32 batches/core,
8 groups of 4 batches):
  - The xl-broadcast is materialized on the HOST and streamed by DMA
    (xlb[g]: [128, 32*512] bf16; row j holds xl[b(g,gi), 2p + j//64, k]
    at col p*512 + gi*128 + k). fmap chunks are produced by DVE-only
    fused tensor_muls (one [128,4096] mul per 8-chunk batch, x0s
    repeated via a 0-stride broadcast AP) at 2x_1P rate. GpSimd is NOT
    used (it shares an SBUF port pair with DVE and would block it).
  - GEMM is group-major: per group 32 chunks x 2 oc matmuls of 512
    moving cols accumulate into 2 PSUM banks; LDWEIGHTS is fully hidden
    behind matmuls by the PE reorder window.
  - A post-schedule pass deletes redundant LDWEIGHTS (resident-identical
    stationary), then verifies every matmul sees the right weights.
  - Bias is added during PSUM evacuation via ScalarE activation.
"""

import sys
import types
import warnings

warnings.filterwarnings("ignore")

import numpy as np
import ml_dtypes

B, M, H, K, O = 256, 64, 64, 128, 256
C = H * M                  # 4096 channels
NCORES = 8
BPC = B // NCORES          # 32 batches per core
GRP = 4                    # batches per group (KB = GRP*K = 512)
NG = BPC // GRP            # 8 groups per core
KB = GRP * K               # 512
NCHUNK = C // 128          # 32 contraction chunks
BCH = 8                    # chunks per xlb DMA batch / fused mul
NB = NCHUNK // BCH         # 4 batches per group

_BF16 = ml_dtypes.bfloat16

LAST_EXEC_NS = None


def _install_ntff_hook():
    try:
        from antenv.axon_hooks import get_axon_ntff_profile_hook  # noqa: F401
        return
    except ImportError:
        pass
    try:
        from trn_agent_boot.trn_boot import _ntff_profile_via_ctypes
        hook = _ntff_profile_via_ctypes('/opt/axon/libaxon_pjrt.so')
    except Exception:
        hook = None
    m = types.ModuleType('antenv.axon_hooks')
    m.get_axon_ntff_profile_hook = lambda: hook
    m.set_axon_ntff_profile_hook = lambda h: None
    sys.modules['antenv.axon_hooks'] = m


_NC_CACHE = {}


def _dedup_ldweights(nc):
    """Delete InstLdweights that reload the identical stationary already
    resident in the PE array (tracked per 32-row strip). Only sync-free
    LDWs are removed. Then verify every matmul sees resident weights."""
    import concourse.mybir as mybir

    def ldw_sig(ap, inst):
        return (str(ap), str(inst.tile_position), str(inst.tile_size),
                str(inst.perf_mode), str(inst.is_transpose))

    def ldw_strips(inst):
        tp = inst.tile_position or (0, 0)
        ts = inst.tile_size
        nrows = ts[0] if ts else 128
        s0 = tp[0] // 32
        s1 = (tp[0] + nrows + 31) // 32
        return range(s0, min(s1, 4))

    removed = kept = 0
    for blk in nc.main_func.blocks:
        il = blk.instructions
        resident = {}
        drop = set()
        for inst in il:
            if getattr(inst, "engine", None) != mybir.EngineType.PE:
                continue
            nm = type(inst).__name__
            if nm == "InstLdweights":
                sig = ldw_sig(inst.ins[0], inst)
                ss = list(ldw_strips(inst))
                if all(resident.get(s) == sig for s in ss) and inst.sync_info is None:
                    drop.add(id(inst))
                    removed += 1
                else:
                    for s in ss:
                        resident[s] = sig
                    kept += 1
            elif nm == "InstMatmult":
                if inst.is_transpose:
                    resident.clear()
            elif nm in ("InstEventSemaphore", "InstDrain", "InstRegisterMove",
                        "InstUnconditionalBranch", "InstNop", "InstMemSet"):
                pass
            else:
                resident.clear()
        if drop:
            il[:] = [i for i in il if id(i) not in drop]

    for blk in nc.main_func.blocks:
        resident = {}
        for inst in blk.instructions:
            if getattr(inst, "engine", None) != mybir.EngineType.PE:
                continue
            nm = type(inst).__name__
            if nm == "InstLdweights":
                sig = ldw_sig(inst.ins[0], inst)
                for s in ldw_strips(inst):
                    resident[s] = sig
            elif nm == "InstMatmult" and not inst.is_transpose:
                wsig = ldw_sig(inst.ins[1], inst)
                ss = list(ldw_strips(inst))
                assert all(resident.get(s) == wsig for s in ss), \
                    f"matmul {inst.name} weights not resident after dedup"
    return removed, kept


def _build_program():
    if "nc" in _NC_CACHE:
        return _NC_CACHE["nc"]
    import concourse.bacc as bacc
    import concourse.tile as tile
    import concourse.mybir as mybir

    dt = mybir.dt
    nc = bacc.Bacc("TRN2", target_bir_lowering=False, debug=False)

    x0s_d = nc.dram_tensor("x0s", [NG, 128, KB], dt.bfloat16, kind="ExternalInput").ap()
    xlb_d = nc.dram_tensor("xlb", [NG, 128, NCHUNK * KB], dt.bfloat16,
                           kind="ExternalInput").ap()
    wt_d = nc.dram_tensor("wt", [128, NCHUNK * O], dt.bfloat16, kind="ExternalInput").ap()
    bias_d = nc.dram_tensor("bias_t", [128, 2], dt.float32, kind="ExternalInput").ap()
    out_d = nc.dram_tensor("out", [NG, 2, 128, KB], dt.bfloat16,
                           kind="ExternalOutput").ap()

    with tile.TileContext(nc) as tc:
        with tc.tile_pool(name="const", bufs=1) as cpool, \
             tc.tile_pool(name="x0p", bufs=2) as x0pool, \
             tc.tile_pool(name="xlbp", bufs=1) as xlbpool, \
             tc.tile_pool(name="fmapp", bufs=1) as fpool, \
             tc.tile_pool(name="outp", bufs=2) as opool, \
             tc.tile_pool(name="psg", bufs=2, space="PSUM") as psgp:

            warm = cpool.tile([128, 256], dt.bfloat16)
            nc.gpsimd.memset(warm[:], 1.0)
            wt = cpool.tile([128, NCHUNK * O], dt.bfloat16)
            bias_t = cpool.tile([128, 2], dt.float32)

            # PE warmup: flips the HAM clock gate to 8/8 while input DMAs run.
            ps_w = psgp.tile([128, KB], dt.float32, name="ps_warm", tag="ps00")
            for wi in range(18):
                nc.tensor.matmul(ps_w[:, 0:256], warm[:, 0:128], warm[:, :],
                                 start=(wi == 0), stop=(wi == 17))

            fm = {}
            x0t = {}

            def produce_steps(g):
                """Yields once per batch; each step DMAs an xlb batch and
                fuses its multiply as one wide DVE op. Group 0 ramps batch
                sizes up so the first GEMM can start early."""
                x0t[g] = x0pool.tile([128, KB], dt.bfloat16,
                                     name=f"x0_{g}", tag=f"x0{g % 2}")
                nc.sync.dma_start(x0t[g][:], x0s_d[g])
                fm[g] = fpool.tile([128, NCHUNK * KB], dt.bfloat16,
                                   name=f"fm_{g}", tag=f"fm{g % 3}")
                xt = xlbpool.tile([128, NCHUNK * KB], dt.bfloat16,
                                  name=f"xlb_{g}", tag=f"xl{g % 2}")
                if g == 0:
                    sizes = [1, 1, 2, 4, 8, 8, 8]
                else:
                    sizes = [BCH] * NB
                p0 = 0
                steps = 0
                for s, bch in enumerate(sizes):
                    if g < 2:
                        nc.sync.dma_start(
                            xt[:, KB * p0:KB * (p0 + bch)],
                            xlb_d[g, :, KB * p0:KB * (p0 + bch)])
                    elif s % 2 == 0:
                        # steady state: 2MB transfers for peak HBM efficiency
                        nc.sync.dma_start(
                            xt[:, KB * p0:KB * (p0 + 2 * BCH)],
                            xlb_d[g, :, KB * p0:KB * (p0 + 2 * BCH)])
                    x0b = x0t[g][:].rearrange("p (o k) -> p o k", o=1) \
                        .broadcast_to([128, bch, KB])
                    nc.vector.tensor_mul(
                        fm[g][:, KB * p0:KB * (p0 + bch)]
                        .rearrange("p (o k) -> p o k", o=bch),
                        xt[:, KB * p0:KB * (p0 + bch)]
                        .rearrange("p (o k) -> p o k", o=bch),
                        x0b)
                    p0 += bch
                    yield
                    steps += 1

            def evac(g, oc, psg_oc):
                osb = opool.tile([128, KB], dt.bfloat16,
                                 name=f"osb_{g}_{oc}", tag=f"osb{g % 2}{oc}")
                nc.scalar.activation(
                    osb[:], psg_oc[:],
                    mybir.ActivationFunctionType.Identity,
                    bias=bias_t[:, oc:oc + 1])
                nc.sync.dma_start(out_d[g, oc], osb[:, :])

            def gemm_steps(g):
                """Yields NB times; step s emits GEMM chunks BCH*s..BCH*(s+1)
                for group g (both oc), evacuating on the last step."""
                psg = {}
                for oc in range(2):
                    psg[oc] = psgp.tile([128, KB], dt.float32,
                                        name=f"psg_{g}_{oc}", tag=f"ps{g % 2}{oc}")
                for s in range(NB):
                    for j in range(BCH):
                        p = BCH * s + j
                        for oc in range(2):
                            nc.tensor.matmul(
                                psg[oc][:],
                                wt[:, O * p + 128 * oc:O * p + 128 * (oc + 1)],
                                fm[g][:, KB * p:KB * (p + 1)],
                                start=(p == 0), stop=(p == NCHUNK - 1))
                    if s == NB - 1:
                        for oc in range(2):
                            evac(g, oc, psg[oc])
                    yield

            def gemm_steps_last(g):
                """oc-sequential variant for the final group: oc0's chain
                completes and evacuates while oc1's chain still runs, so the
                tail only exposes one evac+DMA."""
                psg = {}
                for oc in range(2):
                    psg[oc] = psgp.tile([128, KB], dt.float32,
                                        name=f"psg_{g}_{oc}", tag=f"ps{g % 2}{oc}")
                for oc in range(2):
                    for p in range(NCHUNK):
                        nc.tensor.matmul(
                            psg[oc][:],
                            wt[:, O * p + 128 * oc:O * p + 128 * (oc + 1)],
                            fm[g][:, KB * p:KB * (p + 1)],
                            start=(p == 0), stop=(p == NCHUNK - 1))
                        if oc == 1 and p == 1:
                            yield
                        if oc == 1 and p == NCHUNK // 2:
                            yield
                    evac(g, oc, psg[oc])
                    if oc == 0:
                        yield
                yield

            prev = None
            for g in range(NG):
                prod = produce_steps(g)
                s = 0
                while True:
                    try:
                        next(prod)
                    except StopIteration:
                        break
                    if g == 0 and s == 0:
                        # interleave wt slices proportionally with the xlb
                        # ramp so neither stream starves the first GEMM
                        nc.sync.dma_start(wt[:, 0:4 * O], wt_d[:, 0:4 * O])
                        nc.sync.dma_start(bias_t[:], bias_d[:])
                    if g == 0 and s == 1:
                        nc.sync.dma_start(wt[:, 4 * O:8 * O], wt_d[:, 4 * O:8 * O])
                    if g == 0 and s == 3:
                        nc.sync.dma_start(wt[:, 8 * O:16 * O], wt_d[:, 8 * O:16 * O])
                    if g == 0 and s == 4:
                        nc.sync.dma_start(wt[:, 16 * O:24 * O],
                                          wt_d[:, 16 * O:24 * O])
                    if g == 0 and s == 5:
                        nc.sync.dma_start(wt[:, 24 * O:NCHUNK * O],
                                          wt_d[:, 24 * O:NCHUNK * O])
                    if prev is not None and s < NB:
                        next(prev, None)
                    s += 1
                while prev is not None and next(prev, "end") != "end":
                    pass
                prev = gemm_steps(g) if g < NG - 1 else gemm_steps_last(g)
            for _ in prev:
                pass

    removed, kept = _dedup_ldweights(nc)
    nc.compile()
    _NC_CACHE["nc"] = nc
    _NC_CACHE["ldw"] = (removed, kept)
    return nc


def _host_prep(x0, xl, W, b):
    # x0s[core][g]: [128, KB]  rows j = x0[b, j%64, :], cols gi*K+kk (b = 32c+4g+gi)
    x0g = x0.reshape(NCORES, NG, GRP, M, K).transpose(0, 1, 3, 2, 4) \
        .reshape(NCORES, NG, M, KB)
    x0s = np.concatenate([x0g, x0g], axis=2).astype(_BF16)  # [NC, NG, 128, KB]

    # xlb[core][g]: [128, NCHUNK*KB]; row j=(jh,jr) (jh=j//64), col (p,gi,kk)
    # holds xl[b(g,gi), 2p+jh, kk] (independent of jr).
    xl16 = xl.astype(_BF16)
    xh = xl16.reshape(NCORES, NG, GRP, NCHUNK, 2, K)     # b -> (c,g,gi); h -> (p,jh)
    xh = xh.transpose(0, 1, 3, 4, 2, 5)                  # [NC, NG, p, jh, gi, kk]
    xlb = np.broadcast_to(xh[:, :, :, :, None, :, :],
                          (NCORES, NG, NCHUNK, 2, 64, GRP, K))
    xlb = np.ascontiguousarray(xlb.transpose(0, 1, 3, 4, 2, 5, 6)) \
        .reshape(NCORES, NG, 128, NCHUNK * KB)

    Wm = W[:, :, 0]                        # [O, C]
    wt = np.ascontiguousarray(Wm.T).reshape(NCHUNK, 128, O).transpose(1, 0, 2) \
        .reshape(128, NCHUNK * O).astype(_BF16)   # wt[j, p*O+o] = W[o, 128p+j]

    bias_t = np.ascontiguousarray(b.reshape(2, 128).T.astype(np.float32))  # [128, 2]
    return x0s, xlb, wt, bias_t


def kernel(x0, xl, k, W, b, _trace=False):
    global LAST_EXEC_NS
    _install_ntff_hook()
    import concourse.bass_utils as bass_utils

    x0 = np.asarray(x0, dtype=np.float32)
    xl = np.asarray(xl, dtype=np.float32)
    W = np.asarray(W, dtype=np.float32)
    b = np.asarray(b, dtype=np.float32)

    nc = _build_program()
    x0s, xlb, wt, bias_t = _host_prep(x0, xl, W, b)

    in_maps = [
        {"x0s": np.ascontiguousarray(x0s[c]), "xlb": np.ascontiguousarray(xlb[c]),
         "wt": wt, "bias_t": bias_t}
        for c in range(NCORES)
    ]
    res = bass_utils.run_bass_kernel_spmd(
        nc, in_maps, core_ids=list(range(NCORES)), trace=_trace)
    LAST_EXEC_NS = res.exec_time_ns

    # out_d layout: [NG, 2, o(128), (gi, kk)] -> out[b=(c,g,gi), 128*oc+o, kk]
    out = np.stack([np.asarray(res.results[c]["out"]) for c in range(NCORES)])
    out = out.reshape(NCORES, NG, 2, 128, GRP, K).transpose(0, 1, 4, 2, 3, 5)
    return np.ascontiguousarray(out.reshape(B, O, K).astype(np.float32))
